# revision 4
# baseline (speedup 1.0000x reference)
"""Multi-head attention (B=4, S=2048, E=1024, H=16) on 8 TRN2 NeuronCores.

Sharding: core c -> batch b = c//2, head-half = c%2 (8 heads per core).
Each core computes its heads' QKV + attention + partial output projection
(Megatron-style); the host sums the two partial outputs per batch and adds bo.

Device layout notes (per core):
  hT   [E=1024, S=2048]  hidden transposed (host-side transpose, bf16)
  qT/kT stored [d(pair of heads)=128, S] so scoresT = kT.T-slices @ qT-slices
  scoresT [t_tile=128, s] in PSUM -> exp on ScalarE -> expS bf16 in SBUF
  AV: out[d, s] with lhsT = V[t,d] col-packed 2 heads; rowsum via ones-lhsT
  oT [d(pair)=128, S] feeds the output projection directly as lhsT.
"""

import numpy as np
import ml_dtypes

import concourse.bass as bass
import concourse.mybir as mybir
from concourse import bacc
from concourse.tile import TileContext
from concourse.bass_utils import run_bass_kernel_spmd

BF16 = mybir.dt.bfloat16
F32 = mybir.dt.float32
NP_BF16 = ml_dtypes.bfloat16

B, S, E, H = 4, 2048, 1024, 16
D = E // H  # 64
HPC = H // 2  # heads per core = 8
NPAIR = HPC // 2  # head pairs per core = 4
ET = E // 128  # e contraction tiles = 8
ST = S // 128  # t tiles = 16
SB = S // 512  # s blocks of 512 = 4
EXPF = 0.125  # 1/sqrt(D)

_CACHE = {}


def build_kernel():
    nc = bacc.Bacc()
    hT_d = nc.dram_tensor("hT", [E, S], BF16, kind="ExternalInput")
    wq_d = nc.dram_tensor("wq", [E, HPC * D], BF16, kind="ExternalInput")
    wk_d = nc.dram_tensor("wk", [E, HPC * D], BF16, kind="ExternalInput")
    wv_d = nc.dram_tensor("wv", [E, HPC * D], BF16, kind="ExternalInput")
    wo_d = nc.dram_tensor("wo", [HPC * D, E], BF16, kind="ExternalInput")
    bq_d = nc.dram_tensor("bqT", [128, NPAIR], F32, kind="ExternalInput")
    bk_d = nc.dram_tensor("bkT", [128, NPAIR], F32, kind="ExternalInput")
    bv_d = nc.dram_tensor("bv", [1, HPC * D], BF16, kind="ExternalInput")
    out_d = nc.dram_tensor("out", [S, E], F32, kind="ExternalOutput")

    with TileContext(nc) as tc:
        with (
            tc.tile_pool(name="singles", bufs=1) as singles,
            tc.tile_pool(name="mm", bufs=1, space="PSUM") as psum_mm,
            tc.tile_pool(name="sc", bufs=2, space="PSUM") as psum_sc,
            tc.tile_pool(name="po", bufs=2, space="PSUM") as psum_po,
            tc.tile_pool(name="prs", bufs=1, space="PSUM") as psum_rs,
            tc.tile_pool(name="work", bufs=3) as work,
            tc.tile_pool(name="norm", bufs=2) as norm,
        ):
            # ---- load weights + transposed hidden ----
            hT = singles.tile([128, ET, S], BF16)
            nc.sync.dma_start(out=hT, in_=hT_d.rearrange("(k p) s -> p k s", p=128))
            wq = singles.tile([128, ET, HPC * D], BF16)
            nc.sync.dma_start(out=wq, in_=wq_d.rearrange("(k p) d -> p k d", p=128))
            wk = singles.tile([128, ET, HPC * D], BF16)
            nc.sync.dma_start(out=wk, in_=wk_d.rearrange("(k p) d -> p k d", p=128))
            wv = singles.tile([128, ET, HPC * D], BF16)
            nc.sync.dma_start(out=wv, in_=wv_d.rearrange("(k p) d -> p k d", p=128))
            wo = singles.tile([128, NPAIR, E], BF16)
            nc.sync.dma_start(out=wo, in_=wo_d.rearrange("(k p) n -> p k n", p=128))
            bq = singles.tile([128, NPAIR], F32)
            nc.sync.dma_start(out=bq, in_=bq_d[:, :])
            bk = singles.tile([128, NPAIR], F32)
            nc.sync.dma_start(out=bk, in_=bk_d[:, :])
            bv = singles.tile([1, HPC * D], BF16)
            nc.sync.dma_start(out=bv, in_=bv_d[:, :])

            ones = singles.tile([128, 128], BF16)
            nc.vector.memset(ones, 1.0)

            qT = singles.tile([128, NPAIR, S], BF16)
            kT = singles.tile([128, NPAIR, S], BF16)
            # V with a ones column appended per head: [t, head, 64+1]
            vp = singles.tile([128, ST, HPC, D + 1], BF16)
            nc.vector.memset(vp[:, :, :, D : D + 1], 1.0)
            oT = singles.tile([128, NPAIR, S], BF16)

            # ---- QT / KT projections: qT[d2,s] = (Wq_pair).T @ hiddenT ----
            for w_sb, b_sb, dst in ((wq, bq, qT), (wk, bk, kT)):
                for j in range(NPAIR):
                    for sc in range(SB):
                        ps = psum_mm.tile([128, 512], F32, tag="mm")
                        for k in range(ET):
                            nc.tensor.matmul(
                                ps,
                                lhsT=w_sb[:, k, 128 * j : 128 * (j + 1)],
                                rhs=hT[:, k, 512 * sc : 512 * (sc + 1)],
                                start=(k == 0),
                                stop=(k == ET - 1),
                            )
                        nc.vector.tensor_scalar_add(
                            out=dst[:, j, 512 * sc : 512 * (sc + 1)],
                            in0=ps,
                            scalar1=b_sb[:, j : j + 1],
                        )

            # ---- V projection: V[t, hd] = hiddenT_tile.T @ Wv (+ bv) ----
            for t in range(ST):
                ps = psum_mm.tile([128, 512], F32, tag="mm")
                for k in range(ET):
                    nc.tensor.matmul(
                        ps,
                        lhsT=hT[:, k, 128 * t : 128 * (t + 1)],
                        rhs=wv[:, k, :],
                        start=(k == 0),
                        stop=False,
                    )
                nc.tensor.matmul(
                    ps,
                    lhsT=ones[0:1, 0:128],
                    rhs=bv[0:1, :],
                    start=False,
                    stop=True,
                )
                nc.vector.tensor_copy(
                    out=vp[:, t, :, 0:D],
                    in_=ps.rearrange("p (h u) -> p h u", u=D),
                )

            # ---- attention + output projection ----
            for sc in range(SB):
                s_lo = 512 * sc
                for j in range(NPAIR):
                    po = psum_po.tile([128, 512], F32, tag="po")
                    rs = psum_rs.tile([33, 512], F32, tag="rs")
                    for t in range(ST):
                        ss = psum_sc.tile([128, 1024], F32, tag="ss")
                        # scoresT for the head pair, K=64 row-packed
                        nc.tensor.matmul(
                            ss[:, 0:512],
                            lhsT=kT[0:64, j, 128 * t : 128 * (t + 1)],
                            rhs=qT[0:64, j, s_lo : s_lo + 512],
                            start=True,
                            stop=True,
                        )
                        nc.tensor.matmul(
                            ss[:, 512:1024],
                            lhsT=kT[64:128, j, 128 * t : 128 * (t + 1)],
                            rhs=qT[64:128, j, s_lo : s_lo + 512],
                            start=True,
                            stop=True,
                        )
                        es = work.tile([128, 1024], BF16, tag="es")
                        nc.scalar.activation(
                            es, ss, mybir.ActivationFunctionType.Exp, scale=EXPF
                        )
                        # AV col-packed: head A -> partitions 0:64, B -> 64:128
                        nc.tensor.matmul(
                            po[0:64, :],
                            lhsT=vp[:, t, 2 * j, 0:D],
                            rhs=es[:, 0:512],
                            start=(t == 0),
                            stop=(t == ST - 1),
                        )
                        nc.tensor.matmul(
                            po[64:128, :],
                            lhsT=vp[:, t, 2 * j + 1, 0:D],
                            rhs=es[:, 512:1024],
                            start=(t == 0),
                            stop=(t == ST - 1),
                        )
                        # rowsums (col-packed at positions 0 and 32)
                        nc.tensor.matmul(
                            rs[0:1, :],
                            lhsT=ones[:, 0:1],
                            rhs=es[:, 0:512],
                            start=(t == 0),
                            stop=(t == ST - 1),
                        )
                        nc.tensor.matmul(
                            rs[32:33, :],
                            lhsT=ones[:, 0:1],
                            rhs=es[:, 512:1024],
                            start=(t == 0),
                            stop=(t == ST - 1),
                        )
                    # normalize: oT[:, j, s] = po / rowsum (broadcast over d)
                    # partition_broadcast needs base partition 0 on both sides,
                    # so move head B's rowsum (psum partition 32) to partition 0
                    # via DMA first.
                    rt = norm.tile([33, 512], F32, tag="rt")
                    nc.vector.reciprocal(rt[32:33, :], rs[32:33, :])
                    rr = norm.tile([1, 1024], F32, tag="rr")
                    nc.vector.reciprocal(rr[0:1, 0:512], rs[0:1, :])
                    nc.sync.dma_start(out=rr[0:1, 512:1024], in_=rt[32:33, :])
                    rb = norm.tile([128, 1024], F32, tag="rb")
                    nc.gpsimd.partition_broadcast(rb[:, 0:512], rr[0:1, 0:512])
                    nc.gpsimd.partition_broadcast(rb[:, 512:1024], rr[0:1, 512:1024])
                    nc.vector.tensor_mul(
                        out=oT[0:64, j, s_lo : s_lo + 512],
                        in0=po[0:64, :],
                        in1=rb[0:64, 0:512],
                    )
                    nc.vector.tensor_mul(
                        out=oT[64:128, j, s_lo : s_lo + 512],
                        in0=po[64:128, :],
                        in1=rb[64:128, 512:1024],
                    )
                # output projection for this s block
                for st in range(4 * sc, 4 * (sc + 1)):
                    for n in range(2):
                        ps = psum_mm.tile([128, 512], F32, tag="mm")
                        for j in range(NPAIR):
                            nc.tensor.matmul(
                                ps,
                                lhsT=oT[:, j, 128 * st : 128 * (st + 1)],
                                rhs=wo[:, j, 512 * n : 512 * (n + 1)],
                                start=(j == 0),
                                stop=(j == NPAIR - 1),
                            )
                        ob = work.tile([128, 512], F32, tag="ob")
                        nc.vector.tensor_copy(out=ob, in_=ps)
                        nc.sync.dma_start(
                            out=out_d[128 * st : 128 * (st + 1), 512 * n : 512 * (n + 1)],
                            in_=ob,
                        )

    nc.finalize()
    return nc


def _prep_inputs(hidden_state, Wq, bq, Wk, bk, Wv, bv, Wo, bo):
    """Build the 8 per-core input maps (host-side shard + layout prep)."""
    hidden_state = np.asarray(hidden_state, dtype=np.float32)
    Wq, Wk, Wv = (np.asarray(w, dtype=np.float32) for w in (Wq, Wk, Wv))
    bq, bk, bv = (np.asarray(x, dtype=np.float32) for x in (bq, bk, bv))
    Wo = np.asarray(Wo, dtype=np.float32)

    hTs = [np.ascontiguousarray(hidden_state[b].T).astype(NP_BF16) for b in range(B)]

    halves = []
    for hh in range(2):
        hs = slice(hh * HPC, (hh + 1) * HPC)
        # [H, E, D] -> [E, HPC*D] with head-major columns
        wq_c = np.ascontiguousarray(Wq[hs].transpose(1, 0, 2).reshape(E, HPC * D)).astype(NP_BF16)
        wk_c = np.ascontiguousarray(Wk[hs].transpose(1, 0, 2).reshape(E, HPC * D)).astype(NP_BF16)
        wv_c = np.ascontiguousarray(Wv[hs].transpose(1, 0, 2).reshape(E, HPC * D)).astype(NP_BF16)
        wo_c = np.ascontiguousarray(Wo[hh * HPC * D : (hh + 1) * HPC * D, :]).astype(NP_BF16)
        # per-pair per-partition bias columns: [128, NPAIR]
        bq_c = np.ascontiguousarray(bq[hs].reshape(NPAIR, 128).T).astype(np.float32)
        bk_c = np.ascontiguousarray(bk[hs].reshape(NPAIR, 128).T).astype(np.float32)
        bv_c = np.ascontiguousarray(bv[hs].reshape(1, HPC * D)).astype(NP_BF16)
        halves.append((wq_c, wk_c, wv_c, wo_c, bq_c, bk_c, bv_c))

    in_maps = []
    for c in range(8):
        b, hh = c // 2, c % 2
        wq_c, wk_c, wv_c, wo_c, bq_c, bk_c, bv_c = halves[hh]
        in_maps.append(
            {
                "hT": hTs[b],
                "wq": wq_c,
                "wk": wk_c,
                "wv": wv_c,
                "wo": wo_c,
                "bqT": bq_c,
                "bkT": bk_c,
                "bv": bv_c,
            }
        )
    return in_maps


def run(trace=False, **inputs):
    if "nc" not in _CACHE:
        _CACHE["nc"] = build_kernel()
    nc = _CACHE["nc"]
    in_maps = _prep_inputs(**inputs)
    res = run_bass_kernel_spmd(nc, in_maps, core_ids=list(range(8)), trace=trace)
    bo = np.asarray(inputs["bo"], dtype=np.float32)
    out = np.empty((B, S, E), dtype=np.float32)
    for b in range(B):
        out[b] = res.results[2 * b]["out"] + res.results[2 * b + 1]["out"] + bo
    return out, res


def kernel(**inputs):
    out, _ = run(trace=False, **inputs)
    return out


# revision 6
# speedup vs baseline: 1.1705x; 1.1705x over previous
"""Multi-head attention (B=4, S=2048, E=1024, H=16) on 8 TRN2 NeuronCores.

Sharding: core c -> batch b = c//2, head-half = c%2 (8 heads per core).
Each core computes its heads' QKV + attention + partial output projection
(Megatron-style); the host sums the two partial outputs per batch and adds bo.

Device layout notes (per core):
  hT   [E=1024, S=2048]  hidden transposed (host-side transpose, bf16)
  qT/kT stored [d(pair of heads)=128, S] so scoresT = kT.T-slices @ qT-slices
  scoresT [t_tile=128, s] in PSUM -> exp on ScalarE -> expS bf16 in SBUF
  AV: out[d, s] with lhsT = V[t,d] col-packed 2 heads; rowsum via ones-lhsT
  oT [d(pair)=128, S] feeds the output projection directly as lhsT.

Schedule: pair-outer. Projections for pair j+1 and output projections for
earlier s-blocks are interleaved into the attention t-loops as always-ready
PE filler, keeping the tensor engine dense so the HAM clock gate stays at
full rate.
"""

import numpy as np
import ml_dtypes

import concourse.bass as bass
import concourse.mybir as mybir
from concourse import bacc
from concourse.tile import TileContext
from concourse.bass_utils import run_bass_kernel_spmd

BF16 = mybir.dt.bfloat16
F32 = mybir.dt.float32
NP_BF16 = ml_dtypes.bfloat16

B, S, E, H = 4, 2048, 1024, 16
D = E // H  # 64
HPC = H // 2  # heads per core = 8
NPAIR = HPC // 2  # head pairs per core = 4
ET = E // 128  # e contraction tiles = 8
ST = S // 128  # t tiles = 16
SB = S // 512  # s blocks of 512 = 4
EXPF = 0.125  # 1/sqrt(D)

_CACHE = {}


def build_kernel():
    nc = bacc.Bacc()
    hT_d = nc.dram_tensor("hT", [E, S], BF16, kind="ExternalInput")
    wq_d = nc.dram_tensor("wq", [E, HPC * D], BF16, kind="ExternalInput")
    wk_d = nc.dram_tensor("wk", [E, HPC * D], BF16, kind="ExternalInput")
    wv_d = nc.dram_tensor("wv", [E, HPC * D], BF16, kind="ExternalInput")
    wo_d = nc.dram_tensor("wo", [HPC * D, E], BF16, kind="ExternalInput")
    bq_d = nc.dram_tensor("bqT", [128, NPAIR], F32, kind="ExternalInput")
    bk_d = nc.dram_tensor("bkT", [128, NPAIR], F32, kind="ExternalInput")
    bv_d = nc.dram_tensor("bv", [1, HPC * D], BF16, kind="ExternalInput")
    out_d = nc.dram_tensor("out", [S, E], F32, kind="ExternalOutput")

    with TileContext(nc) as tc:
        with (
            tc.tile_pool(name="singles", bufs=1) as singles,
            tc.tile_pool(name="mm", bufs=2, space="PSUM") as psum_mm,
            tc.tile_pool(name="sc", bufs=2, space="PSUM") as psum_sc,
            tc.tile_pool(name="po", bufs=1, space="PSUM") as psum_po,
            tc.tile_pool(name="prs", bufs=1, space="PSUM") as psum_rs,
            tc.tile_pool(name="work", bufs=3) as work,
            tc.tile_pool(name="norm", bufs=2) as norm,
        ):
            # ---- load weights + transposed hidden ----
            hT = singles.tile([128, ET, S], BF16)
            nc.sync.dma_start(out=hT, in_=hT_d.rearrange("(k p) s -> p k s", p=128))
            wq = singles.tile([128, ET, HPC * D], BF16)
            nc.sync.dma_start(out=wq, in_=wq_d.rearrange("(k p) d -> p k d", p=128))
            wk = singles.tile([128, ET, HPC * D], BF16)
            nc.sync.dma_start(out=wk, in_=wk_d.rearrange("(k p) d -> p k d", p=128))
            wv = singles.tile([128, ET, HPC * D], BF16)
            nc.sync.dma_start(out=wv, in_=wv_d.rearrange("(k p) d -> p k d", p=128))
            wo = singles.tile([128, NPAIR, E], BF16)
            nc.sync.dma_start(out=wo, in_=wo_d.rearrange("(k p) n -> p k n", p=128))
            bq = singles.tile([128, NPAIR], F32)
            nc.sync.dma_start(out=bq, in_=bq_d[:, :])
            bk = singles.tile([128, NPAIR], F32)
            nc.sync.dma_start(out=bk, in_=bk_d[:, :])
            bv = singles.tile([1, HPC * D], BF16)
            nc.sync.dma_start(out=bv, in_=bv_d[:, :])

            ones = singles.tile([128, 128], BF16)
            nc.vector.memset(ones, 1.0)

            qT = singles.tile([128, NPAIR, S], BF16)
            kT = singles.tile([128, NPAIR, S], BF16)
            # V with a ones column appended per head: [t, head, 64+1]
            vp = singles.tile([128, ST, HPC, D + 1], BF16)
            nc.vector.memset(vp[:, :, :, D : D + 1], 1.0)
            oT = singles.tile([128, NPAIR, S], BF16)

            def qk_proj_unit(j, sc):
                """One psum group: qT and kT chunk for (pair j, s chunk sc)."""

                def emit():
                    for w_sb, b_sb, dst in ((wq, bq, qT), (wk, bk, kT)):
                        ps = psum_mm.tile([128, 512], F32, tag="mm")
                        for k in range(ET):
                            nc.tensor.matmul(
                                ps,
                                lhsT=w_sb[:, k, 128 * j : 128 * (j + 1)],
                                rhs=hT[:, k, 512 * sc : 512 * (sc + 1)],
                                start=(k == 0),
                                stop=(k == ET - 1),
                            )
                        nc.vector.tensor_scalar_add(
                            out=dst[:, j, 512 * sc : 512 * (sc + 1)],
                            in0=ps,
                            scalar1=b_sb[:, j : j + 1],
                        )

                return emit

            def v_proj_unit(j, tpair):
                """V proj for pair j, two t tiles (N=128 cols of wv each)."""

                def emit():
                    for t in (2 * tpair, 2 * tpair + 1):
                        ps = psum_mm.tile([128, 128], F32, tag="mm")
                        for k in range(ET):
                            nc.tensor.matmul(
                                ps,
                                lhsT=hT[:, k, 128 * t : 128 * (t + 1)],
                                rhs=wv[:, k, 128 * j : 128 * (j + 1)],
                                start=(k == 0),
                                stop=False,
                            )
                        nc.tensor.matmul(
                            ps,
                            lhsT=ones[0:1, 0:128],
                            rhs=bv[0:1, 128 * j : 128 * (j + 1)],
                            start=False,
                            stop=True,
                        )
                        nc.vector.tensor_copy(
                            out=vp[:, t, 2 * j : 2 * j + 2, 0:D],
                            in_=ps.rearrange("p (h u) -> p h u", u=D),
                        )

                return emit

            def oproj_unit(st, n):
                """Output projection chunk: s tile st, col chunk n."""

                def emit():
                    ps = psum_mm.tile([128, 512], F32, tag="mm")
                    for j in range(NPAIR):
                        nc.tensor.matmul(
                            ps,
                            lhsT=oT[:, j, 128 * st : 128 * (st + 1)],
                            rhs=wo[:, j, 512 * n : 512 * (n + 1)],
                            start=(j == 0),
                            stop=(j == NPAIR - 1),
                        )
                    ob = work.tile([128, 512], F32, tag="ob")
                    nc.vector.tensor_copy(out=ob, in_=ps)
                    nc.sync.dma_start(
                        out=out_d[128 * st : 128 * (st + 1), 512 * n : 512 * (n + 1)],
                        in_=ob,
                    )

                return emit

            # ---- phase A: projections for pair 0 ----
            for sc in range(SB):
                qk_proj_unit(0, sc)()
            for tp in range(ST // 2):
                v_proj_unit(0, tp)()

            # ---- attention, pair-outer; filler interleaved ----
            # filler positions within each 16-iteration t loop
            FILL_T = (1, 3, 5, 7, 9, 11, 13, 15)

            for j in range(NPAIR):
                # build this pair's filler inventory (ready-to-run PE work)
                filler = []
                if j + 1 < NPAIR:
                    for sc in range(SB):
                        filler.append(qk_proj_unit(j + 1, sc))
                    for tp in range(ST // 2):
                        filler.append(v_proj_unit(j + 1, tp))
                for sc in range(SB):
                    s_lo = 512 * sc
                    if j == NPAIR - 1 and sc > 0:
                        # previous s block's output projection is now ready
                        for st in range(4 * (sc - 1), 4 * sc):
                            for n in range(2):
                                filler.append(oproj_unit(st, n))
                    po = psum_po.tile([128, 512], F32, tag="po")
                    rs = psum_rs.tile([33, 512], F32, tag="rs")
                    nfill = max(0, min(len(filler), (len(filler) + SB - 1 - sc) // (SB - sc)))
                    fill_at = set(FILL_T[:nfill])
                    for t in range(ST):
                        ss = psum_sc.tile([128, 1024], F32, tag="ss")
                        # scoresT for the head pair, K=64 row-packed
                        nc.tensor.matmul(
                            ss[:, 0:512],
                            lhsT=kT[0:64, j, 128 * t : 128 * (t + 1)],
                            rhs=qT[0:64, j, s_lo : s_lo + 512],
                            start=True,
                            stop=True,
                        )
                        nc.tensor.matmul(
                            ss[:, 512:1024],
                            lhsT=kT[64:128, j, 128 * t : 128 * (t + 1)],
                            rhs=qT[64:128, j, s_lo : s_lo + 512],
                            start=True,
                            stop=True,
                        )
                        es = work.tile([128, 1024], BF16, tag="es")
                        nc.scalar.activation(
                            es, ss, mybir.ActivationFunctionType.Exp, scale=EXPF
                        )
                        # AV col-packed: head A -> partitions 0:64, B -> 64:128
                        nc.tensor.matmul(
                            po[0:64, :],
                            lhsT=vp[:, t, 2 * j, 0:D],
                            rhs=es[:, 0:512],
                            start=(t == 0),
                            stop=(t == ST - 1),
                        )
                        nc.tensor.matmul(
                            po[64:128, :],
                            lhsT=vp[:, t, 2 * j + 1, 0:D],
                            rhs=es[:, 512:1024],
                            start=(t == 0),
                            stop=(t == ST - 1),
                        )
                        # rowsums (col-packed at partition positions 0 and 32)
                        nc.tensor.matmul(
                            rs[0:1, :],
                            lhsT=ones[:, 0:1],
                            rhs=es[:, 0:512],
                            start=(t == 0),
                            stop=(t == ST - 1),
                        )
                        nc.tensor.matmul(
                            rs[32:33, :],
                            lhsT=ones[:, 0:1],
                            rhs=es[:, 512:1024],
                            start=(t == 0),
                            stop=(t == ST - 1),
                        )
                        if t in fill_at and filler:
                            filler.pop(0)()
                    # copy accumulators out of PSUM fast, then normalize from
                    # SBUF so the PSUM banks free up for the next iteration.
                    poc = work.tile([128, 512], F32, tag="poc")
                    nc.vector.tensor_copy(out=poc, in_=po)
                    rt = norm.tile([33, 512], F32, tag="rt")
                    nc.vector.reciprocal(rt[32:33, :], rs[32:33, :])
                    rr = norm.tile([1, 1024], F32, tag="rr")
                    nc.vector.reciprocal(rr[0:1, 0:512], rs[0:1, :])
                    # partition_broadcast needs base partition 0 on both sides:
                    # move head B's reciprocal to partition 0 via sbuf DMA.
                    nc.sync.dma_start(out=rr[0:1, 512:1024], in_=rt[32:33, :])
                    rb = norm.tile([128, 1024], F32, tag="rb")
                    nc.gpsimd.partition_broadcast(rb[:, 0:512], rr[0:1, 0:512])
                    nc.gpsimd.partition_broadcast(rb[:, 512:1024], rr[0:1, 512:1024])
                    nc.vector.tensor_mul(
                        out=oT[0:64, j, s_lo : s_lo + 512],
                        in0=poc[0:64, :],
                        in1=rb[0:64, 0:512],
                    )
                    nc.vector.tensor_mul(
                        out=oT[64:128, j, s_lo : s_lo + 512],
                        in0=poc[64:128, :],
                        in1=rb[64:128, 512:1024],
                    )
                # drain any leftover filler before moving to the next pair
                while filler:
                    filler.pop(0)()

            # ---- tail: last s block's output projection ----
            for st in range(4 * (SB - 1), 4 * SB):
                for n in range(2):
                    oproj_unit(st, n)()

    nc.finalize()
    return nc


def _prep_inputs(hidden_state, Wq, bq, Wk, bk, Wv, bv, Wo, bo):
    """Build the 8 per-core input maps (host-side shard + layout prep)."""
    hidden_state = np.asarray(hidden_state, dtype=np.float32)
    Wq, Wk, Wv = (np.asarray(w, dtype=np.float32) for w in (Wq, Wk, Wv))
    bq, bk, bv = (np.asarray(x, dtype=np.float32) for x in (bq, bk, bv))
    Wo = np.asarray(Wo, dtype=np.float32)

    hTs = [np.ascontiguousarray(hidden_state[b].T).astype(NP_BF16) for b in range(B)]

    halves = []
    for hh in range(2):
        hs = slice(hh * HPC, (hh + 1) * HPC)
        # [H, E, D] -> [E, HPC*D] with head-major columns
        wq_c = np.ascontiguousarray(Wq[hs].transpose(1, 0, 2).reshape(E, HPC * D)).astype(NP_BF16)
        wk_c = np.ascontiguousarray(Wk[hs].transpose(1, 0, 2).reshape(E, HPC * D)).astype(NP_BF16)
        wv_c = np.ascontiguousarray(Wv[hs].transpose(1, 0, 2).reshape(E, HPC * D)).astype(NP_BF16)
        wo_c = np.ascontiguousarray(Wo[hh * HPC * D : (hh + 1) * HPC * D, :]).astype(NP_BF16)
        # per-pair per-partition bias columns: [128, NPAIR]
        bq_c = np.ascontiguousarray(bq[hs].reshape(NPAIR, 128).T).astype(np.float32)
        bk_c = np.ascontiguousarray(bk[hs].reshape(NPAIR, 128).T).astype(np.float32)
        bv_c = np.ascontiguousarray(bv[hs].reshape(1, HPC * D)).astype(NP_BF16)
        halves.append((wq_c, wk_c, wv_c, wo_c, bq_c, bk_c, bv_c))

    in_maps = []
    for c in range(8):
        b, hh = c // 2, c % 2
        wq_c, wk_c, wv_c, wo_c, bq_c, bk_c, bv_c = halves[hh]
        in_maps.append(
            {
                "hT": hTs[b],
                "wq": wq_c,
                "wk": wk_c,
                "wv": wv_c,
                "wo": wo_c,
                "bqT": bq_c,
                "bkT": bk_c,
                "bv": bv_c,
            }
        )
    return in_maps


def run(trace=False, **inputs):
    if "nc" not in _CACHE:
        _CACHE["nc"] = build_kernel()
    nc = _CACHE["nc"]
    in_maps = _prep_inputs(**inputs)
    res = run_bass_kernel_spmd(nc, in_maps, core_ids=list(range(8)), trace=trace)
    bo = np.asarray(inputs["bo"], dtype=np.float32)
    out = np.empty((B, S, E), dtype=np.float32)
    for b in range(B):
        out[b] = res.results[2 * b]["out"] + res.results[2 * b + 1]["out"] + bo
    return out, res


def kernel(**inputs):
    out, _ = run(trace=False, **inputs)
    return out


# revision 8
# speedup vs baseline: 1.5180x; 1.2969x over previous
"""Multi-head attention (B=4, S=2048, E=1024, H=16) on 8 TRN2 NeuronCores.

Sharding: core c -> batch b = c//2, head-half = c%2 (8 heads per core).
Each core computes its heads' QKV + attention + partial output projection
(Megatron-style); the host sums the two partial outputs per batch and adds bo.

Device layout notes (per core):
  hT   [E=1024, S=2048]  hidden transposed (host-side transpose, bf16)
  qT/kT stored [d(pair of heads)=128, S] so scoresT = kT.T-slices @ qT-slices
  scoresT [t_tile=128, s] in PSUM -> exp on ScalarE -> expS bf16 in SBUF
  AV: out[d, s] with lhsT = V[t,d] col-packed 2 heads; rowsum via ones-lhsT
  oT [d(pair)=128, S] feeds the output projection directly as lhsT.

Schedule: pair-outer. Projections for pair j+1 and output projections for
earlier s-blocks are interleaved into the attention t-loops as always-ready
PE filler, keeping the tensor engine dense so the HAM clock gate stays at
full rate.
"""

import numpy as np
import ml_dtypes

import concourse.bass as bass
import concourse.mybir as mybir
from concourse import bacc
from concourse.tile import TileContext
from concourse.bass_utils import run_bass_kernel_spmd

BF16 = mybir.dt.bfloat16
F32 = mybir.dt.float32
NP_BF16 = ml_dtypes.bfloat16

B, S, E, H = 4, 2048, 1024, 16
D = E // H  # 64
HPC = H // 2  # heads per core = 8
NPAIR = HPC // 2  # head pairs per core = 4
ET = E // 128  # e contraction tiles = 8
ST = S // 128  # t tiles = 16
SB = S // 512  # s blocks of 512 = 4
EXPF = 0.125  # 1/sqrt(D)

_CACHE = {}


def build_kernel():
    nc = bacc.Bacc()
    hT_d = nc.dram_tensor("hT", [E, S], BF16, kind="ExternalInput")
    wq_d = nc.dram_tensor("wq", [E, HPC * D], BF16, kind="ExternalInput")
    wk_d = nc.dram_tensor("wk", [E, HPC * D], BF16, kind="ExternalInput")
    wv_d = nc.dram_tensor("wv", [E, HPC * D], BF16, kind="ExternalInput")
    wo_d = nc.dram_tensor("wo", [HPC * D, E], BF16, kind="ExternalInput")
    bq_d = nc.dram_tensor("bqT", [128, NPAIR], F32, kind="ExternalInput")
    bk_d = nc.dram_tensor("bkT", [128, NPAIR], F32, kind="ExternalInput")
    bv_d = nc.dram_tensor("bv", [1, HPC * D], BF16, kind="ExternalInput")
    out_d = nc.dram_tensor("out", [S, E], F32, kind="ExternalOutput")

    with TileContext(nc) as tc:
        with (
            tc.tile_pool(name="singles", bufs=1) as singles,
            tc.tile_pool(name="mm", bufs=2, space="PSUM") as psum_mm,
            tc.tile_pool(name="sc", bufs=2, space="PSUM") as psum_sc,
            tc.tile_pool(name="po", bufs=1, space="PSUM") as psum_po,
            tc.tile_pool(name="prs", bufs=1, space="PSUM") as psum_rs,
            tc.tile_pool(name="work", bufs=3) as work,
            tc.tile_pool(name="norm", bufs=2) as norm,
        ):
            # ---- load weights + transposed hidden ----
            hT = singles.tile([128, ET, S], BF16)
            nc.sync.dma_start(out=hT, in_=hT_d.rearrange("(k p) s -> p k s", p=128))
            wq = singles.tile([128, ET, HPC * D], BF16)
            nc.sync.dma_start(out=wq, in_=wq_d.rearrange("(k p) d -> p k d", p=128))
            wk = singles.tile([128, ET, HPC * D], BF16)
            nc.sync.dma_start(out=wk, in_=wk_d.rearrange("(k p) d -> p k d", p=128))
            wv = singles.tile([128, ET, HPC * D], BF16)
            nc.sync.dma_start(out=wv, in_=wv_d.rearrange("(k p) d -> p k d", p=128))
            wo = singles.tile([128, NPAIR, E], BF16)
            nc.sync.dma_start(out=wo, in_=wo_d.rearrange("(k p) n -> p k n", p=128))
            bq = singles.tile([128, NPAIR], F32)
            nc.sync.dma_start(out=bq, in_=bq_d[:, :])
            bk = singles.tile([128, NPAIR], F32)
            nc.sync.dma_start(out=bk, in_=bk_d[:, :])
            bv = singles.tile([1, HPC * D], BF16)
            nc.sync.dma_start(out=bv, in_=bv_d[:, :])

            ones = singles.tile([128, 128], BF16)
            nc.vector.memset(ones, 1.0)

            qT = singles.tile([128, NPAIR, S], BF16)
            kT = singles.tile([128, NPAIR, S], BF16)
            # V with a ones column appended per head: [t, head, 64+1]
            vp = singles.tile([128, ST, HPC, D + 1], BF16)
            nc.vector.memset(vp[:, :, :, D : D + 1], 1.0)
            oT = singles.tile([128, NPAIR, S], BF16)

            def qk_proj_unit(j, sc):
                """One psum group: qT and kT chunk for (pair j, s chunk sc)."""

                def emit():
                    for w_sb, b_sb, dst in ((wq, bq, qT), (wk, bk, kT)):
                        ps = psum_mm.tile([128, 512], F32, tag="mm")
                        for k in range(ET):
                            nc.tensor.matmul(
                                ps,
                                lhsT=w_sb[:, k, 128 * j : 128 * (j + 1)],
                                rhs=hT[:, k, 512 * sc : 512 * (sc + 1)],
                                start=(k == 0),
                                stop=(k == ET - 1),
                            )
                        nc.vector.tensor_scalar_add(
                            out=dst[:, j, 512 * sc : 512 * (sc + 1)],
                            in0=ps,
                            scalar1=b_sb[:, j : j + 1],
                        )

                return emit

            def v_proj_unit(j, tpair):
                """V proj for pair j, two t tiles (N=128 cols of wv each)."""

                def emit():
                    for t in (2 * tpair, 2 * tpair + 1):
                        ps = psum_mm.tile([128, 128], F32, tag="mm")
                        for k in range(ET):
                            nc.tensor.matmul(
                                ps,
                                lhsT=hT[:, k, 128 * t : 128 * (t + 1)],
                                rhs=wv[:, k, 128 * j : 128 * (j + 1)],
                                start=(k == 0),
                                stop=False,
                            )
                        nc.tensor.matmul(
                            ps,
                            lhsT=ones[0:1, 0:128],
                            rhs=bv[0:1, 128 * j : 128 * (j + 1)],
                            start=False,
                            stop=True,
                        )
                        nc.vector.tensor_copy(
                            out=vp[:, t, 2 * j : 2 * j + 2, 0:D],
                            in_=ps.rearrange("p (h u) -> p h u", u=D),
                        )

                return emit

            def oproj_unit(st, n):
                """Output projection chunk: s tile st, col chunk n."""

                def emit():
                    ps = psum_mm.tile([128, 512], F32, tag="mm")
                    for j in range(NPAIR):
                        nc.tensor.matmul(
                            ps,
                            lhsT=oT[:, j, 128 * st : 128 * (st + 1)],
                            rhs=wo[:, j, 512 * n : 512 * (n + 1)],
                            start=(j == 0),
                            stop=(j == NPAIR - 1),
                        )
                    ob = work.tile([128, 512], F32, tag="ob")
                    nc.vector.tensor_copy(out=ob, in_=ps)
                    nc.sync.dma_start(
                        out=out_d[128 * st : 128 * (st + 1), 512 * n : 512 * (n + 1)],
                        in_=ob,
                    )

                return emit

            # ---- phase A: projections for pair 0 ----
            for sc in range(SB):
                qk_proj_unit(0, sc)()
            for tp in range(ST // 2):
                v_proj_unit(0, tp)()

            # ---- attention: flat software-pipelined stream over (j, sc, t).
            # scores lead AV/rowsum by one slot so the tensor engine always
            # has work during each exp, and exp(t+1) can start the moment
            # exp(t) finishes. Projection / output-projection units are
            # interleaved as always-ready PE filler.
            slots = [(j, sc, t) for j in range(NPAIR) for sc in range(SB) for t in range(ST)]
            po_tiles = {}
            rs_tiles = {}
            filler = []

            def emit_scores(j, sc, t):
                if t == 0:
                    po_tiles[(j, sc)] = psum_po.tile([128, 512], F32, tag="po", name="po")
                    rs_tiles[(j, sc)] = psum_rs.tile([33, 512], F32, tag="rs", name="rs")
                ss = psum_sc.tile([128, 1024], F32, tag="ss")
                nc.tensor.matmul(
                    ss[:, 0:512],
                    lhsT=kT[0:64, j, 128 * t : 128 * (t + 1)],
                    rhs=qT[0:64, j, 512 * sc : 512 * (sc + 1)],
                    start=True,
                    stop=True,
                )
                nc.tensor.matmul(
                    ss[:, 512:1024],
                    lhsT=kT[64:128, j, 128 * t : 128 * (t + 1)],
                    rhs=qT[64:128, j, 512 * sc : 512 * (sc + 1)],
                    start=True,
                    stop=True,
                )
                return ss

            def emit_av_rs(j, sc, t, es):
                po = po_tiles[(j, sc)]
                rs = rs_tiles[(j, sc)]
                # AV col-packed: head A -> partitions 0:64, B -> 64:128
                nc.tensor.matmul(
                    po[0:64, :],
                    lhsT=vp[:, t, 2 * j, 0:D],
                    rhs=es[:, 0:512],
                    start=(t == 0),
                    stop=(t == ST - 1),
                )
                nc.tensor.matmul(
                    po[64:128, :],
                    lhsT=vp[:, t, 2 * j + 1, 0:D],
                    rhs=es[:, 512:1024],
                    start=(t == 0),
                    stop=(t == ST - 1),
                )
                # rowsums (col-packed at partition positions 0 and 32)
                nc.tensor.matmul(
                    rs[0:1, :],
                    lhsT=ones[:, 0:1],
                    rhs=es[:, 0:512],
                    start=(t == 0),
                    stop=(t == ST - 1),
                )
                nc.tensor.matmul(
                    rs[32:33, :],
                    lhsT=ones[:, 0:1],
                    rhs=es[:, 512:1024],
                    start=(t == 0),
                    stop=(t == ST - 1),
                )

            def emit_normalize(j, sc):
                # copy accumulators out of PSUM immediately (frees the banks
                # for the next iteration); the slow reciprocal/broadcast chain
                # then runs from SBUF concurrently with the next t loop.
                po = po_tiles.pop((j, sc))
                rs = rs_tiles.pop((j, sc))
                s_lo = 512 * sc
                poc = work.tile([128, 512], F32, tag="poc")
                nc.vector.tensor_copy(out=poc, in_=po)
                rsc = work.tile([33, 512], F32, tag="rsc")
                nc.vector.tensor_copy(out=rsc, in_=rs)
                rt = norm.tile([33, 512], F32, tag="rt")
                nc.vector.reciprocal(rt[32:33, :], rsc[32:33, :])
                rr = norm.tile([1, 1024], F32, tag="rr")
                nc.vector.reciprocal(rr[0:1, 0:512], rsc[0:1, :])
                # partition_broadcast needs base partition 0 on both sides:
                # move head B's reciprocal to partition 0 via sbuf DMA.
                nc.sync.dma_start(out=rr[0:1, 512:1024], in_=rt[32:33, :])
                rb = norm.tile([128, 1024], F32, tag="rb")
                nc.gpsimd.partition_broadcast(rb[:, 0:512], rr[0:1, 0:512])
                nc.gpsimd.partition_broadcast(rb[:, 512:1024], rr[0:1, 512:1024])
                nc.vector.tensor_mul(
                    out=oT[0:64, j, s_lo : s_lo + 512],
                    in0=poc[0:64, :],
                    in1=rb[0:64, 0:512],
                )
                nc.vector.tensor_mul(
                    out=oT[64:128, j, s_lo : s_lo + 512],
                    in0=poc[64:128, :],
                    in1=rb[64:128, 512:1024],
                )

            ss_cur = emit_scores(*slots[0])
            for n, (j, sc, t) in enumerate(slots):
                es = work.tile([128, 1024], BF16, tag="es")
                nc.scalar.activation(
                    es, ss_cur, mybir.ActivationFunctionType.Exp, scale=EXPF
                )
                # refill the filler inventory as units become ready
                if t == 0 and sc == 0 and j + 1 < NPAIR:
                    for sc2 in range(SB):
                        filler.append(qk_proj_unit(j + 1, sc2))
                    for tp in range(ST // 2):
                        filler.append(v_proj_unit(j + 1, tp))
                if t == 0 and j == NPAIR - 1 and sc > 0:
                    for st in range(4 * (sc - 1), 4 * sc):
                        for n2 in range(2):
                            filler.append(oproj_unit(st, n2))
                if n + 1 < len(slots):
                    ss_cur = emit_scores(*slots[n + 1])
                emit_av_rs(j, sc, t, es)
                if t % 2 == 1 and filler:
                    filler.pop(0)()
                if t == ST - 1:
                    emit_normalize(j, sc)

            # ---- tail: remaining filler + last s block's output projection ----
            while filler:
                filler.pop(0)()
            for st in range(4 * (SB - 1), 4 * SB):
                for n in range(2):
                    oproj_unit(st, n)()

    nc.finalize()
    return nc


def _prep_inputs(hidden_state, Wq, bq, Wk, bk, Wv, bv, Wo, bo):
    """Build the 8 per-core input maps (host-side shard + layout prep)."""
    hidden_state = np.asarray(hidden_state, dtype=np.float32)
    Wq, Wk, Wv = (np.asarray(w, dtype=np.float32) for w in (Wq, Wk, Wv))
    bq, bk, bv = (np.asarray(x, dtype=np.float32) for x in (bq, bk, bv))
    Wo = np.asarray(Wo, dtype=np.float32)

    hTs = [np.ascontiguousarray(hidden_state[b].T).astype(NP_BF16) for b in range(B)]

    halves = []
    for hh in range(2):
        hs = slice(hh * HPC, (hh + 1) * HPC)
        # [H, E, D] -> [E, HPC*D] with head-major columns
        wq_c = np.ascontiguousarray(Wq[hs].transpose(1, 0, 2).reshape(E, HPC * D)).astype(NP_BF16)
        wk_c = np.ascontiguousarray(Wk[hs].transpose(1, 0, 2).reshape(E, HPC * D)).astype(NP_BF16)
        wv_c = np.ascontiguousarray(Wv[hs].transpose(1, 0, 2).reshape(E, HPC * D)).astype(NP_BF16)
        wo_c = np.ascontiguousarray(Wo[hh * HPC * D : (hh + 1) * HPC * D, :]).astype(NP_BF16)
        # per-pair per-partition bias columns: [128, NPAIR]
        bq_c = np.ascontiguousarray(bq[hs].reshape(NPAIR, 128).T).astype(np.float32)
        bk_c = np.ascontiguousarray(bk[hs].reshape(NPAIR, 128).T).astype(np.float32)
        bv_c = np.ascontiguousarray(bv[hs].reshape(1, HPC * D)).astype(NP_BF16)
        halves.append((wq_c, wk_c, wv_c, wo_c, bq_c, bk_c, bv_c))

    in_maps = []
    for c in range(8):
        b, hh = c // 2, c % 2
        wq_c, wk_c, wv_c, wo_c, bq_c, bk_c, bv_c = halves[hh]
        in_maps.append(
            {
                "hT": hTs[b],
                "wq": wq_c,
                "wk": wk_c,
                "wv": wv_c,
                "wo": wo_c,
                "bqT": bq_c,
                "bkT": bk_c,
                "bv": bv_c,
            }
        )
    return in_maps


def run(trace=False, **inputs):
    if "nc" not in _CACHE:
        _CACHE["nc"] = build_kernel()
    nc = _CACHE["nc"]
    in_maps = _prep_inputs(**inputs)
    res = run_bass_kernel_spmd(nc, in_maps, core_ids=list(range(8)), trace=trace)
    bo = np.asarray(inputs["bo"], dtype=np.float32)
    out = np.empty((B, S, E), dtype=np.float32)
    for b in range(B):
        out[b] = res.results[2 * b]["out"] + res.results[2 * b + 1]["out"] + bo
    return out, res


def kernel(**inputs):
    out, _ = run(trace=False, **inputs)
    return out
